# revision 29
# baseline (speedup 1.0000x reference)
"""v5: GQA attention, 1 kv-head x 4 q-heads x B=2 per core, SEQ=2048.

Per-core dataflow:
  - Startup fast path: K b0 (both halves) + Q h0 lo-half loaded as f32 via SP
    HWDGE into qf32 staging, PE f32-transposes into free psum (slot0/1 +
    banks 6/7), DVE cast-copies psum f32 -> KT0/QT0 f16. Avoids the Pool
    SWDGE prep + xbar chain on the critical path to S(0)/S(1).
  - All other loads: Pool (gpsimd) casting-DMAs f32 DRAM -> f16 qnat16
    (3-buf rotation), then SP xbar DMA transposes -> KT/QT
    (out[:, t, :] = in[:, 128t:+128]^T per 128x128 tile).
  - V: Pool casting-DMA f32 -> f16 vt3 (128 d cols + ones col).
  - PE: per q-block pair p (128): S = KT^T @ QT into psum slot g%3 (1024
    f32 cols per group g=2p,2p+1), O = PT^T @ [V|1] into opsum (129 cols,
    banks 6/7); warmup matmuls interleaved at start to hold the p-state.
  - exp: ACT does cols 0:640 of each group; DVE does cols 640:1024 via
    1-pass int16 Schraudolph tensor_scalar (f32 psum -> i16 bitcast f16 PT).
  - DVE: reciprocal of ones-col denominator, scale O into OS (f32).
  - SP stores OS -> DRAM in quarter-head chunks (last quarter split in two).
"""
import numpy as np
import concourse.bass as bass
from concourse import mybir
from contextlib import ExitStack

F32 = mybir.dt.float32
F16 = mybir.dt.float16
I16 = mybir.dt.int16
EXP = mybir.ActivationFunctionType.Exp
SCALE = float(1.0 / np.sqrt(128.0))
LN2 = float(np.log(2.0))
A16 = (1 << 10) / LN2 * SCALE
B16 = 15.0 * (1 << 10) - 61.0

N_CORES = 8
N_WARM = 40                   # warmups before first PE transpose
N_WARM2 = 11                  # fill between tr groups
AW = 640                      # ACT exp cols per group; DVE does 1024-AW
LABELS = {}


def _lab(inst, label):
    try:
        LABELS[inst.ins.name] = label
    except Exception:
        pass
    return inst


def build_attention_nc(SEQ=2048, B=2, G=4):
    D = 128
    T = SEQ // 128            # 16 k/q tiles per head
    H = B * G                 # 8 (b, g) heads per core
    NPH = T                   # pairs (q-blocks) per head
    NPAIR = H * NPH           # 128
    NW = NPAIR + 2            # windows (O lags S by 2 pairs)
    assert T == 16 and H == 8 and B == 2

    nc = bass.Bass()
    q_ext = nc.declare_dram_parameter("query", [SEQ, B, G, D], F32, isOutput=False)
    k_ext = nc.declare_dram_parameter("key", [SEQ, B, D], F32, isOutput=False)
    v_ext = nc.declare_dram_parameter("value", [SEQ, B, D], F32, isOutput=False)
    o_ext = nc.declare_dram_parameter("out", [SEQ, B, G, D], F32, isOutput=True)

    # loads in first-use order: K(b0), Q(h0..h3), K(b1), Q(h4..h7)
    loads = [("K", 0, None)] + [("Q", 0, g) for g in range(G)]
    loads += [("K", 1, None)] + [("Q", 1, g) for g in range(G)]
    NL = len(loads)           # 10

    def q_load_index(h):
        b, g = divmod(h, G)
        return b * (G + 1) + 1 + g

    # fast-path f32 SP loads: (dst qf32 idx, load, t0, t1)
    FAST = [(0, 0, 0, 8), (1, 1, 0, 8), (2, 0, 8, 16)]
    # Pool/xbar chunks: Q h0 hi-half, then loads 2..9 whole
    pool_chunks = [(1, 8, 16)] + [(i, 0, 16) for i in range(2, NL)]
    NPC = len(pool_chunks)    # 9

    def pc_of_load(i):        # xbar chunk index of load i (i >= 2)
        return i - 1

    def q_pc(h, qc):          # xbar chunk for S's Q tile; None if fast path
        if h == 0:
            return 0 if qc >= 8 else None
        return pc_of_load(q_load_index(h))

    def k_pc(b):              # xbar chunk for K; None if fast path (b0)
        return None if b == 0 else pc_of_load(5)

    # ---------------- PE walk (sem_pe: f32 trs + S/O matmuls) -----------
    pe = 24                   # 3 groups of 8 f32 transposes
    pe_after_S = {}
    pe_after_O = {}
    pe += 8
    pe_after_S[0] = pe
    pe += 8
    pe_after_S[1] = pe
    for w in range(1, NW):
        if w < NPAIR:
            pe += 8
            pe_after_S[2 * w] = pe
        if w >= 2:
            pe += 16
            pe_after_O[w - 2] = pe
        if w < NPAIR:
            pe += 8
            pe_after_S[2 * w + 1] = pe

    # ---------------- DVE walk (sem_dve counter) ------------------------
    dve = 3                   # cpK0, cpQ0, cpK1 prepended
    CP_K0, CP_Q0, CP_K1 = 1, 2, 3
    dexp_done = {}
    recips_done = {}
    mults_done = {}
    dve_ops = []
    for w in range(NW):
        if w < NPAIR:
            dve += 1
            dexp_done[2 * w] = dve
            dve_ops.append(("dexp", 2 * w))
        if w >= 2:
            p = w - 2
            dve += 1
            recips_done[p] = dve
            dve_ops.append(("recip", p))
            dve += 1
            mults_done[p] = dve
            dve_ops.append(("mult", p))
        if w < NPAIR:
            dve += 1
            dexp_done[2 * w + 1] = dve
            dve_ops.append(("dexp", 2 * w + 1))

    # ---------------- tensors ----------------
    bias0 = nc.alloc_sbuf_tensor("bias0", [128, 1], F32)
    scr = nc.alloc_sbuf_tensor("scr", [128, 1], F32)
    warm = nc.alloc_sbuf_tensor("warm", [128, 128], F16)
    ident = nc.alloc_sbuf_tensor("ident", [128, 128], F32)
    qf32 = [nc.alloc_sbuf_tensor(f"qf32_{j}", [128, 1024], F32)
            for j in range(3)]
    qnat16 = [nc.alloc_sbuf_tensor(f"qnat16_{i}", [128, T * 128], F16)
              for i in range(3)]
    KT = [nc.alloc_sbuf_tensor(f"KT{b}", [128, T * 128], F16) for b in range(B)]
    QT = [nc.alloc_sbuf_tensor(f"QT{h}", [128, T * 128], F16) for h in range(H)]
    VT = [nc.alloc_sbuf_tensor(f"VT{b}", [128, T * 132], F16) for b in range(B)]
    PT = [nc.alloc_sbuf_tensor(f"PT{s}", [128, 2048], F16) for s in range(3)]
    rsb = [nc.alloc_sbuf_tensor(f"rsb{s}", [128, 1], F32) for s in range(2)]
    OS = [nc.alloc_sbuf_tensor(f"OS{s}", [128, T * 128], F32) for s in range(2)]
    psum = nc.alloc_psum_tensor("psum", [128, 4096], F32)

    pv = psum[:, :].rearrange("p (s c) -> p s c", c=1024)
    PTI = [PT[s][:, :].bitcast(I16) for s in range(3)]

    def spsum_mm(slot, ki):
        return psum[:, slot * 1024 + ki * 128: slot * 1024 + (ki + 1) * 128]

    def opsum(buf):
        off = 3072 + buf * 512
        return psum[:, off:off + 129]

    # f32 transpose staging areas (psum col base per fast-path group)
    TR_BASE = {0: 3072, 1: 0, 2: 1024}   # K-lo: banks 6/7, Q-lo: slot0, K-hi: slot1

    # ACT exp col width per group: lighter for the last 2 pairs (tail latency)
    def aw_of(g):
        return 256 if g >= 2 * NPAIR - 4 else AW

    # O matmul k-tile order: ACT-produced PT cols first, DVE-produced last.
    # (kts_of returns (order, odd_act_wait_index, dexp_wait_index))
    def kts_of(p):
        if p >= NPAIR - 2:
            return ([0, 1, 8, 9, 2, 3, 4, 5, 6, 7, 10, 11, 12, 13, 14, 15],
                    2, 4)
        return ([0, 1, 2, 3, 4, 8, 9, 10, 11, 12, 5, 6, 7, 13, 14, 15],
                5, 10)

    with ExitStack() as ctx:
        sem_pe = ctx.enter_context(nc.semaphore("sem_pe"))
        sem_act = ctx.enter_context(nc.semaphore("sem_act"))
        sem_dve = ctx.enter_context(nc.semaphore("sem_dve"))
        sem_pool = ctx.enter_context(nc.semaphore("sem_pool"))
        sem_ldf = [ctx.enter_context(nc.semaphore(f"sem_ldf{j}"))
                   for j in range(3)]
        sem_ld = [ctx.enter_context(nc.semaphore(f"sem_ld{c}"))
                  for c in range(NPC)]
        sem_tr = [ctx.enter_context(nc.semaphore(f"sem_tr{c}"))
                  for c in range(NPC)]
        sem_out = [ctx.enter_context(nc.semaphore(f"sem_out{h}"))
                   for h in range(H)]
        sem_v = [ctx.enter_context(nc.semaphore(f"sem_v{b}")) for b in range(B)]
        block = ctx.enter_context(nc.Block())

        def ld_src(i):
            kind, b, g = loads[i]
            return k_ext[:, b, :] if kind == "K" else q_ext[:, b, g, :]

        @block.gpsimd
        def _(gp):
            # sem_pool: 1 warm, 2 ident0, 3 ident, 4 bias0, 5 vt0, 6 vt1
            nc.gpsimd.memset(warm[:], 0.0).then_inc(sem_pool)
            nc.gpsimd.memset(ident[:], 0.0).then_inc(sem_pool)
            nc.gpsimd.wait_ge(sem_pool, 2)
            nc.gpsimd.affine_select(
                out=ident[:], in_=ident[:],
                compare_op=mybir.AluOpType.not_equal, fill=1.0,
                base=0, pattern=[[-1, 128]], channel_multiplier=1,
            ).then_inc(sem_pool)

            def emit_load(pc):
                i, t0, t1 = pool_chunks[pc]
                if i >= 4:
                    nc.gpsimd.wait_ge(sem_tr[pc_of_load(i - 3)], 16)
                src = ld_src(i).rearrange("(t p) d -> p t d", p=128)
                dst = qnat16[i % 3][:].rearrange("p (t d) -> p t d", d=128)
                _lab(nc.gpsimd.dma_start(
                    out=dst[:, t0:t1, :], in_=src[:, t0:t1, :],
                ), f"L(pc{pc},l{i})").then_inc(sem_ld[pc], 16)

            def emit_vload(b):
                vt3 = VT[b][:].rearrange("p (t c) -> p t c", c=132)
                _lab(nc.gpsimd.dma_start(
                    out=vt3[:, :, 0:128],
                    in_=v_ext[:, b, :].rearrange("(t p) d -> p t d", p=128),
                ), f"V({b})").then_inc(sem_v[b], 16)

            emit_load(0)          # Q h0 t8-15
            emit_vload(0)
            nc.gpsimd.memset(bias0[:], 0.0).then_inc(sem_pool)
            for b in range(B):
                vt3 = VT[b][:].rearrange("p (t c) -> p t c", c=132)
                nc.gpsimd.memset(vt3[:, :, 128:129], 1.0).then_inc(sem_pool)
            emit_load(1)          # Q h1
            emit_vload(1)
            for pc in range(2, NPC):
                emit_load(pc)

        @block.sync
        def _(sync):
            # fast-path f32 loads
            for j, i, t0, t1 in FAST:
                src = ld_src(i).rearrange("(t p) d -> p t d", p=128)
                dst = qf32[j][:].rearrange("p (t d) -> p t d", d=128)
                _lab(nc.sync.dma_start(
                    out=dst[:, 0:8, :], in_=src[:, t0:t1, :],
                ), f"Lf{j}").then_inc(sem_ldf[j], 16)

            # xbar transposes for pool chunks
            for pc in range(NPC):
                i, t0, t1 = pool_chunks[pc]
                nc.sync.wait_ge(sem_ld[pc], 16)
                kind, b, g = loads[i]
                tt = KT[b] if kind == "K" else QT[b * G + g]
                dst = tt[:].rearrange("p (t d) -> p t d", d=128)
                _lab(nc.sync.dma_start_transpose(
                    dst[:, t0:t1, :],
                    qnat16[i % 3][:, t0 * 128:t1 * 128],
                ), f"T(pc{pc},l{i})").then_inc(sem_tr[pc], 16)

            # output stores, quarter-head granularity (last quarter split)
            def store(h, t0, t1, p_end):
                b, g = divmod(h, G)
                oh = o_ext[:, b, g, :].rearrange("(t p) d -> p t d", p=128)
                osh = OS[h % 2][:].rearrange("p (t d) -> p t d", d=128)
                nc.sync.wait_ge(sem_dve, mults_done[p_end])
                _lab(nc.sync.dma_start(
                    out=oh[:, t0:t1, :], in_=osh[:, t0:t1, :],
                ), f"st(h{h},t{t0})").then_inc(sem_out[h], 16)

            for h in range(H):
                for q in range(4):
                    if h == H - 1 and q == 3:
                        store(h, 12, 15, h * NPH + 14)
                        store(h, 15, 16, h * NPH + 15)
                    else:
                        store(h, 4 * q, 4 * q + 4, h * NPH + 4 * q + 3)
            for h in range(H):
                nc.sync.wait_ge(sem_out[h], 80 if h == H - 1 else 64)

        @block.tensor
        def _(te):
            nc.tensor.wait_ge(sem_pool, 1)
            nwarm = [0]

            def emit_warm(n):
                for _ in range(n):
                    nwarm[0] += 1
                    _lab(nc.tensor.matmul(
                        psum[:, 2048:2176], warm[:], warm[:],
                        start=True, stop=True, skip_group_check=True,
                    ), f"warm{nwarm[0]}")

            def emit_ftr(j):
                if j == 0:
                    nc.tensor.wait_ge(sem_pool, 3)   # ident ready
                nc.tensor.wait_ge(sem_ldf[j], 16)
                base = TR_BASE[j]
                for t in range(8):
                    _lab(nc.tensor.transpose(
                        psum[:, base + t * 128: base + (t + 1) * 128],
                        qf32[j][:, t * 128:(t + 1) * 128], ident[:],
                    ), f"ftr(j{j},t{t})").then_inc(sem_pe)

            done_pc = set()
            last_dve = [0]

            def pc_wait(pc):
                if pc is not None and pc not in done_pc:
                    done_pc.add(pc)
                    nc.tensor.wait_ge(sem_tr[pc], 16)

            def dve_wait(val):
                if val > last_dve[0]:
                    last_dve[0] = val
                    nc.tensor.wait_ge(sem_dve, val)

            def emit_S(g):
                p = g >> 1
                h = p // NPH
                slot = g % 3
                kp = g & 1
                b = h // G
                qc = p % NPH
                pc_wait(k_pc(b))
                pc_wait(q_pc(h, qc))
                if g == 0:
                    dve_wait(CP_Q0)          # covers cpK0 too
                elif g == 1:
                    dve_wait(CP_K1)
                if g >= 3:
                    dve_wait(dexp_done[g - 3])
                for ki in range(8):
                    kt = kp * 8 + ki
                    inst = nc.tensor.matmul(
                        spsum_mm(slot, ki),
                        KT[b][:, kt * 128:(kt + 1) * 128],
                        QT[h][:, qc * 128:(qc + 1) * 128],
                        start=True, stop=True, skip_group_check=True,
                    )
                    if ki == 0 and g >= 3:
                        inst._wait_ge(sem_act, g - 2)
                    _lab(inst, f"S(g{g},ki{ki})")
                    inst.then_inc(sem_pe)

            def emit_O(p):
                h = p // NPH
                b = h // G
                buf = p % 2
                if p == 0 or p == G * NPH:
                    nc.tensor.wait_ge(sem_v[b], 16)
                    nc.tensor.wait_ge(sem_pool, 5 + b)
                if p >= 2:
                    dve_wait(mults_done[p - 2])   # opsum buf reuse
                vt3 = VT[b][:].rearrange("p (t c) -> p t c", c=132)
                kts, odd_act_i, dve_i = kts_of(p)
                for i, kt in enumerate(kts):
                    half = kt // 8
                    ki = kt % 8
                    inst = nc.tensor.matmul(
                        opsum(buf),
                        PT[p % 3][:, half * 1024 + ki * 128:
                                  half * 1024 + (ki + 1) * 128],
                        vt3[:, kt, 0:129],
                        start=(i == 0), stop=(i == len(kts) - 1),
                        skip_group_check=True,
                    )
                    if i == 0:
                        inst._wait_ge(sem_act, 2 * p + 1)
                    if i == odd_act_i:
                        inst._wait_ge(sem_act, 2 * p + 2)
                    if i == dve_i:
                        inst._wait_ge(sem_dve, dexp_done[2 * p + 1])
                    _lab(inst, f"O(p{p},kt{kt})")
                    inst.then_inc(sem_pe)

            emit_warm(N_WARM)
            emit_ftr(0)           # K b0 t0-7 -> banks 6/7
            emit_warm(N_WARM2)
            emit_ftr(1)           # Q h0 t0-7 -> slot 0
            emit_warm(N_WARM2)
            emit_ftr(2)           # K b0 t8-15 -> slot 1
            emit_S(0)
            emit_S(1)
            for w in range(1, NW):
                if w < NPAIR:
                    emit_S(2 * w)
                if w >= 2:
                    emit_O(w - 2)
                if w < NPAIR:
                    emit_S(2 * w + 1)

        @block.scalar
        def _(sc):
            nc.scalar.wait_ge(sem_pool, 4)
            nc.scalar.activation(                  # preload Exp table
                out=scr[:, 0:1], in_=bias0[:, 0:1],
                func=EXP, bias=bias0[:, 0:1], scale=1.0,
            )
            for g in range(2 * NPAIR):
                p = g >> 1
                half = g & 1
                slot = g % 3
                aw = aw_of(g)
                ov = PT[p % 3][:, :].rearrange("p (s c) -> p s c", c=1024)
                _lab(nc.scalar.activation(
                    out=ov[:, half:half + 1, 0:aw],
                    in_=pv[:, slot:slot + 1, 0:aw],
                    func=EXP, bias=bias0[:, 0:1], scale=SCALE,
                )._wait_ge(sem_pe, pe_after_S[g]),
                    f"exp(g{g})").then_inc(sem_act)

        @block.vector
        def _(ve):
            # fast-path cast copies psum f32 -> KT/QT f16
            for label, pe_val, dst, base in [
                ("cpK0", 8, KT[0][:, 0:1024], TR_BASE[0]),
                ("cpQ0", 16, QT[0][:, 0:1024], TR_BASE[1]),
                ("cpK1", 24, KT[0][:, 1024:2048], TR_BASE[2]),
            ]:
                nc.vector.wait_ge(sem_pe, pe_val)
                _lab(nc.vector.tensor_copy(
                    dst, psum[:, base:base + 1024]), label).then_inc(sem_dve)

            def emit_dexp(g):
                p = g >> 1
                half = g & 1
                slot = g % 3
                aw = aw_of(g)
                nc.vector.wait_ge(sem_pe, pe_after_S[g])
                _lab(nc.vector.tensor_scalar(
                    PTI[p % 3][:, half * 1024 + aw:half * 1024 + 1024],
                    pv[:, slot, aw:1024],
                    A16, B16, op0=mybir.AluOpType.mult,
                    op1=mybir.AluOpType.add,
                ), f"dexp(g{g})").then_inc(sem_dve)

            def emit_recip(p):
                buf = p % 2
                nc.vector.wait_ge(sem_pe, pe_after_O[p])
                if p >= 2:
                    nc.vector.wait_ge(sem_dve, mults_done[p - 2])
                _lab(nc.vector.reciprocal(
                    rsb[buf][:, 0:1], opsum(buf)[:, 128:129]),
                    f"recip({p})").then_inc(sem_dve)

            def emit_mult(p):
                h = p // NPH
                qc = p % NPH
                buf = p % 2
                nc.vector.wait_ge(sem_dve, recips_done[p])
                if qc == 0 and h >= 2:
                    nc.vector.wait_ge(sem_out[h - 2], 64)
                _lab(nc.vector.tensor_scalar(
                    OS[h % 2][:, qc * 128:(qc + 1) * 128],
                    opsum(buf)[:, 0:128],
                    rsb[buf][:, 0:1],
                    None,
                    op0=mybir.AluOpType.mult,
                ), f"mult({p})").then_inc(sem_dve)

            for op in dve_ops:
                if op[0] == "dexp":
                    emit_dexp(op[1])
                elif op[0] == "recip":
                    emit_recip(op[1])
                else:
                    emit_mult(op[1])

    return nc


_NC = None


def _get_nc():
    global _NC
    if _NC is None:
        _NC = build_attention_nc(2048, 2, 4)
    return _NC


def kernel(query, key, value):
    from concourse.bass_utils import run_bass_kernel_spmd

    query = np.ascontiguousarray(query, dtype=np.float32)
    key = np.ascontiguousarray(key, dtype=np.float32)
    value = np.ascontiguousarray(value, dtype=np.float32)
    G = query.shape[2] // key.shape[2]
    nc = _get_nc()
    in_maps = []
    for c in range(N_CORES):
        in_maps.append({
            "query": np.ascontiguousarray(query[:, :, c * G:(c + 1) * G, :]),
            "key": np.ascontiguousarray(key[:, :, c, :]),
            "value": np.ascontiguousarray(value[:, :, c, :]),
        })
    res = run_bass_kernel_spmd(nc, in_maps, list(range(N_CORES)))
    out = np.empty_like(query)
    for c in range(N_CORES):
        out[:, :, c * G:(c + 1) * G, :] = res.results[c]["out"]
    return out


# revision 30
# speedup vs baseline: 1.0043x; 1.0043x over previous
"""v5: GQA attention, 1 kv-head x 4 q-heads x B=2 per core, SEQ=2048.

Per-core dataflow:
  - Startup fast path: K b0 (both halves) + Q h0 lo-half loaded as f32 via SP
    HWDGE into qf32 staging, PE f32-transposes into free psum (slot0/1 +
    banks 6/7), DVE cast-copies psum f32 -> KT0/QT0 f16. Avoids the Pool
    SWDGE prep + xbar chain on the critical path to S(0)/S(1).
  - All other loads: Pool (gpsimd) casting-DMAs f32 DRAM -> f16 qnat16
    (3-buf rotation), then SP xbar DMA transposes -> KT/QT
    (out[:, t, :] = in[:, 128t:+128]^T per 128x128 tile).
  - V: Pool casting-DMA f32 -> f16 vt3 (128 d cols + ones col).
  - PE: per q-block pair p (128): S = KT^T @ QT into psum slot g%3 (1024
    f32 cols per group g=2p,2p+1), O = PT^T @ [V|1] into opsum (129 cols,
    banks 6/7); warmup matmuls interleaved at start to hold the p-state.
  - exp: ACT does cols 0:640 of each group; DVE does cols 640:1024 via
    1-pass int16 Schraudolph tensor_scalar (f32 psum -> i16 bitcast f16 PT).
  - DVE: reciprocal of ones-col denominator, scale O into OS (f32).
  - SP stores OS -> DRAM in quarter-head chunks (last quarter split in two).
"""
import numpy as np
import concourse.bass as bass
from concourse import mybir
from contextlib import ExitStack

F32 = mybir.dt.float32
F16 = mybir.dt.float16
I16 = mybir.dt.int16
EXP = mybir.ActivationFunctionType.Exp
SCALE = float(1.0 / np.sqrt(128.0))
LN2 = float(np.log(2.0))
A16 = (1 << 10) / LN2 * SCALE
B16 = 15.0 * (1 << 10) - 61.0

N_CORES = 8
N_WARM = 26                   # warmups before first PE transpose
N_WARM2 = 9                  # fill between tr groups
AW = 640                      # ACT exp cols per group; DVE does 1024-AW
LABELS = {}


def _lab(inst, label):
    try:
        LABELS[inst.ins.name] = label
    except Exception:
        pass
    return inst


def build_attention_nc(SEQ=2048, B=2, G=4):
    D = 128
    T = SEQ // 128            # 16 k/q tiles per head
    H = B * G                 # 8 (b, g) heads per core
    NPH = T                   # pairs (q-blocks) per head
    NPAIR = H * NPH           # 128
    NW = NPAIR + 2            # windows (O lags S by 2 pairs)
    assert T == 16 and H == 8 and B == 2

    nc = bass.Bass()
    q_ext = nc.declare_dram_parameter("query", [SEQ, B, G, D], F32, isOutput=False)
    k_ext = nc.declare_dram_parameter("key", [SEQ, B, D], F32, isOutput=False)
    v_ext = nc.declare_dram_parameter("value", [SEQ, B, D], F32, isOutput=False)
    o_ext = nc.declare_dram_parameter("out", [SEQ, B, G, D], F32, isOutput=True)

    # loads in first-use order: K(b0), Q(h0..h3), K(b1), Q(h4..h7)
    loads = [("K", 0, None)] + [("Q", 0, g) for g in range(G)]
    loads += [("K", 1, None)] + [("Q", 1, g) for g in range(G)]
    NL = len(loads)           # 10

    def q_load_index(h):
        b, g = divmod(h, G)
        return b * (G + 1) + 1 + g

    # fast-path f32 SP loads: (dst qf32 idx, load, t0, t1)
    FAST = [(0, 0, 0, 8), (1, 1, 0, 8), (2, 0, 8, 16)]
    # Pool/xbar chunks: Q h0 hi-half, then loads 2..9 whole
    pool_chunks = [(1, 8, 16)] + [(i, 0, 16) for i in range(2, NL)]
    NPC = len(pool_chunks)    # 9

    def pc_of_load(i):        # xbar chunk index of load i (i >= 2)
        return i - 1

    def q_pc(h, qc):          # xbar chunk for S's Q tile; None if fast path
        if h == 0:
            return 0 if qc >= 8 else None
        return pc_of_load(q_load_index(h))

    def k_pc(b):              # xbar chunk for K; None if fast path (b0)
        return None if b == 0 else pc_of_load(5)

    # ---------------- PE walk (sem_pe: f32 trs + S/O matmuls) -----------
    pe = 24                   # 3 groups of 8 f32 transposes
    pe_after_S = {}
    pe_after_O = {}
    pe += 8
    pe_after_S[0] = pe
    pe += 8
    pe_after_S[1] = pe
    for w in range(1, NW):
        if w < NPAIR:
            pe += 8
            pe_after_S[2 * w] = pe
        if w >= 2:
            pe += 16
            pe_after_O[w - 2] = pe
        if w < NPAIR:
            pe += 8
            pe_after_S[2 * w + 1] = pe

    # ---------------- DVE walk (sem_dve counter) ------------------------
    dve = 4                   # cpK0, cpQ0, cpK1a, cpK1b prepended
    CP_K0, CP_Q0, CP_K1A, CP_K1B = 1, 2, 3, 4
    dexp_done = {}
    recips_done = {}
    mults_done = {}
    dve_ops = []
    for w in range(NW):
        if w < NPAIR:
            dve += 1
            dexp_done[2 * w] = dve
            dve_ops.append(("dexp", 2 * w))
        if w >= 2:
            p = w - 2
            dve += 1
            recips_done[p] = dve
            dve_ops.append(("recip", p))
            dve += 1
            mults_done[p] = dve
            dve_ops.append(("mult", p))
        if w < NPAIR:
            dve += 1
            dexp_done[2 * w + 1] = dve
            dve_ops.append(("dexp", 2 * w + 1))

    # ---------------- tensors ----------------
    bias0 = nc.alloc_sbuf_tensor("bias0", [128, 1], F32)
    scr = nc.alloc_sbuf_tensor("scr", [128, 1], F32)
    warm = nc.alloc_sbuf_tensor("warm", [128, 128], F16)
    ident = nc.alloc_sbuf_tensor("ident", [128, 128], F32)
    qf32 = [nc.alloc_sbuf_tensor(f"qf32_{j}", [128, 1024], F32)
            for j in range(3)]
    qnat16 = [nc.alloc_sbuf_tensor(f"qnat16_{i}", [128, T * 128], F16)
              for i in range(3)]
    KT = [nc.alloc_sbuf_tensor(f"KT{b}", [128, T * 128], F16) for b in range(B)]
    QT = [nc.alloc_sbuf_tensor(f"QT{h}", [128, T * 128], F16) for h in range(H)]
    VT = [nc.alloc_sbuf_tensor(f"VT{b}", [128, T * 132], F16) for b in range(B)]
    PT = [nc.alloc_sbuf_tensor(f"PT{s}", [128, 2048], F16) for s in range(3)]
    rsb = [nc.alloc_sbuf_tensor(f"rsb{s}", [128, 1], F32) for s in range(2)]
    OS = [nc.alloc_sbuf_tensor(f"OS{s}", [128, T * 128], F32) for s in range(2)]
    psum = nc.alloc_psum_tensor("psum", [128, 4096], F32)

    pv = psum[:, :].rearrange("p (s c) -> p s c", c=1024)
    PTI = [PT[s][:, :].bitcast(I16) for s in range(3)]

    def spsum_mm(slot, ki):
        return psum[:, slot * 1024 + ki * 128: slot * 1024 + (ki + 1) * 128]

    def opsum(buf):
        off = 3072 + buf * 512
        return psum[:, off:off + 129]

    # f32 transpose staging areas (psum col base per fast-path group)
    TR_BASE = {0: 3072, 1: 0, 2: 1024}   # K-lo: banks 6/7, Q-lo: slot0, K-hi: slot1

    # ACT exp col width per group: lighter for the last 2 pairs (tail latency)
    def aw_of(g):
        return 256 if g >= 2 * NPAIR - 4 else AW

    # O matmul k-tile order: ACT-produced PT cols first, DVE-produced last.
    # (kts_of returns (order, odd_act_wait_index, dexp_wait_index))
    def kts_of(p):
        if p >= NPAIR - 2:
            return ([0, 1, 8, 9, 2, 3, 4, 5, 6, 7, 10, 11, 12, 13, 14, 15],
                    2, 4)
        return ([0, 1, 2, 3, 4, 8, 9, 10, 11, 12, 5, 6, 7, 13, 14, 15],
                5, 10)

    with ExitStack() as ctx:
        sem_pe = ctx.enter_context(nc.semaphore("sem_pe"))
        sem_act = ctx.enter_context(nc.semaphore("sem_act"))
        sem_dve = ctx.enter_context(nc.semaphore("sem_dve"))
        sem_pool = ctx.enter_context(nc.semaphore("sem_pool"))
        sem_ldf = [ctx.enter_context(nc.semaphore(f"sem_ldf{j}"))
                   for j in range(3)]
        sem_ld = [ctx.enter_context(nc.semaphore(f"sem_ld{c}"))
                  for c in range(NPC)]
        sem_tr = [ctx.enter_context(nc.semaphore(f"sem_tr{c}"))
                  for c in range(NPC)]
        sem_out = [ctx.enter_context(nc.semaphore(f"sem_out{h}"))
                   for h in range(H)]
        sem_v = [ctx.enter_context(nc.semaphore(f"sem_v{b}")) for b in range(B)]
        block = ctx.enter_context(nc.Block())

        def ld_src(i):
            kind, b, g = loads[i]
            return k_ext[:, b, :] if kind == "K" else q_ext[:, b, g, :]

        @block.gpsimd
        def _(gp):
            # sem_pool: 1 warm, 2 ident0, 3 ident, 4 bias0, 5 vt0, 6 vt1
            nc.gpsimd.memset(warm[:], 0.0).then_inc(sem_pool)
            nc.gpsimd.memset(ident[:], 0.0).then_inc(sem_pool)
            nc.gpsimd.wait_ge(sem_pool, 2)
            nc.gpsimd.affine_select(
                out=ident[:], in_=ident[:],
                compare_op=mybir.AluOpType.not_equal, fill=1.0,
                base=0, pattern=[[-1, 128]], channel_multiplier=1,
            ).then_inc(sem_pool)

            def emit_load(pc):
                i, t0, t1 = pool_chunks[pc]
                if i >= 4:
                    nc.gpsimd.wait_ge(sem_tr[pc_of_load(i - 3)], 16)
                src = ld_src(i).rearrange("(t p) d -> p t d", p=128)
                dst = qnat16[i % 3][:].rearrange("p (t d) -> p t d", d=128)
                _lab(nc.gpsimd.dma_start(
                    out=dst[:, t0:t1, :], in_=src[:, t0:t1, :],
                ), f"L(pc{pc},l{i})").then_inc(sem_ld[pc], 16)

            def emit_vload(b):
                vt3 = VT[b][:].rearrange("p (t c) -> p t c", c=132)
                _lab(nc.gpsimd.dma_start(
                    out=vt3[:, :, 0:128],
                    in_=v_ext[:, b, :].rearrange("(t p) d -> p t d", p=128),
                ), f"V({b})").then_inc(sem_v[b], 16)

            emit_vload(0)
            emit_load(0)          # Q h0 t8-15
            nc.gpsimd.memset(bias0[:], 0.0).then_inc(sem_pool)
            for b in range(B):
                vt3 = VT[b][:].rearrange("p (t c) -> p t c", c=132)
                nc.gpsimd.memset(vt3[:, :, 128:129], 1.0).then_inc(sem_pool)
            emit_load(1)          # Q h1
            emit_vload(1)
            for pc in range(2, NPC):
                emit_load(pc)

        @block.sync
        def _(sync):
            # fast-path f32 loads
            for j, i, t0, t1 in FAST:
                src = ld_src(i).rearrange("(t p) d -> p t d", p=128)
                dst = qf32[j][:].rearrange("p (t d) -> p t d", d=128)
                _lab(nc.sync.dma_start(
                    out=dst[:, 0:8, :], in_=src[:, t0:t1, :],
                ), f"Lf{j}").then_inc(sem_ldf[j], 16)

            # xbar transposes for pool chunks
            for pc in range(NPC):
                i, t0, t1 = pool_chunks[pc]
                nc.sync.wait_ge(sem_ld[pc], 16)
                kind, b, g = loads[i]
                tt = KT[b] if kind == "K" else QT[b * G + g]
                dst = tt[:].rearrange("p (t d) -> p t d", d=128)
                _lab(nc.sync.dma_start_transpose(
                    dst[:, t0:t1, :],
                    qnat16[i % 3][:, t0 * 128:t1 * 128],
                ), f"T(pc{pc},l{i})").then_inc(sem_tr[pc], 16)

            # output stores, quarter-head granularity (last quarter split)
            def store(h, t0, t1, p_end):
                b, g = divmod(h, G)
                oh = o_ext[:, b, g, :].rearrange("(t p) d -> p t d", p=128)
                osh = OS[h % 2][:].rearrange("p (t d) -> p t d", d=128)
                nc.sync.wait_ge(sem_dve, mults_done[p_end])
                _lab(nc.sync.dma_start(
                    out=oh[:, t0:t1, :], in_=osh[:, t0:t1, :],
                ), f"st(h{h},t{t0})").then_inc(sem_out[h], 16)

            for h in range(H):
                for q in range(4):
                    if h == H - 1 and q == 3:
                        store(h, 12, 15, h * NPH + 14)
                        store(h, 15, 16, h * NPH + 15)
                    else:
                        store(h, 4 * q, 4 * q + 4, h * NPH + 4 * q + 3)
            for h in range(H):
                nc.sync.wait_ge(sem_out[h], 80 if h == H - 1 else 64)

        @block.tensor
        def _(te):
            nc.tensor.wait_ge(sem_pool, 1)
            nwarm = [0]

            def emit_warm(n):
                for _ in range(n):
                    nwarm[0] += 1
                    _lab(nc.tensor.matmul(
                        psum[:, 2048:2176], warm[:], warm[:],
                        start=True, stop=True, skip_group_check=True,
                    ), f"warm{nwarm[0]}")

            def emit_ftr(j):
                if j == 0:
                    nc.tensor.wait_ge(sem_pool, 3)   # ident ready
                nc.tensor.wait_ge(sem_ldf[j], 16)
                base = TR_BASE[j]
                for t in range(8):
                    _lab(nc.tensor.transpose(
                        psum[:, base + t * 128: base + (t + 1) * 128],
                        qf32[j][:, t * 128:(t + 1) * 128], ident[:],
                    ), f"ftr(j{j},t{t})").then_inc(sem_pe)

            done_pc = set()
            last_dve = [0]

            def pc_wait(pc):
                if pc is not None and pc not in done_pc:
                    done_pc.add(pc)
                    nc.tensor.wait_ge(sem_tr[pc], 16)

            def dve_wait(val):
                if val > last_dve[0]:
                    last_dve[0] = val
                    nc.tensor.wait_ge(sem_dve, val)

            def emit_S(g):
                p = g >> 1
                h = p // NPH
                slot = g % 3
                kp = g & 1
                b = h // G
                qc = p % NPH
                pc_wait(k_pc(b))
                pc_wait(q_pc(h, qc))
                if g == 0:
                    dve_wait(CP_Q0)          # covers cpK0 too
                if g >= 3:
                    dve_wait(dexp_done[g - 3])
                for ki in range(8):
                    kt = kp * 8 + ki
                    inst = nc.tensor.matmul(
                        spsum_mm(slot, ki),
                        KT[b][:, kt * 128:(kt + 1) * 128],
                        QT[h][:, qc * 128:(qc + 1) * 128],
                        start=True, stop=True, skip_group_check=True,
                    )
                    if ki == 0 and g >= 3:
                        inst._wait_ge(sem_act, g - 2)
                    if g == 1 and ki == 0:
                        inst._wait_ge(sem_dve, CP_K1A)
                    if g == 1 and ki == 4:
                        inst._wait_ge(sem_dve, CP_K1B)
                    _lab(inst, f"S(g{g},ki{ki})")
                    inst.then_inc(sem_pe)

            def emit_O(p):
                h = p // NPH
                b = h // G
                buf = p % 2
                if p == 0 or p == G * NPH:
                    nc.tensor.wait_ge(sem_v[b], 16)
                    nc.tensor.wait_ge(sem_pool, 5 + b)
                if p >= 2:
                    dve_wait(mults_done[p - 2])   # opsum buf reuse
                vt3 = VT[b][:].rearrange("p (t c) -> p t c", c=132)
                kts, odd_act_i, dve_i = kts_of(p)
                for i, kt in enumerate(kts):
                    half = kt // 8
                    ki = kt % 8
                    inst = nc.tensor.matmul(
                        opsum(buf),
                        PT[p % 3][:, half * 1024 + ki * 128:
                                  half * 1024 + (ki + 1) * 128],
                        vt3[:, kt, 0:129],
                        start=(i == 0), stop=(i == len(kts) - 1),
                        skip_group_check=True,
                    )
                    if i == 0:
                        inst._wait_ge(sem_act, 2 * p + 1)
                    if i == odd_act_i:
                        inst._wait_ge(sem_act, 2 * p + 2)
                    if i == dve_i:
                        inst._wait_ge(sem_dve, dexp_done[2 * p + 1])
                    _lab(inst, f"O(p{p},kt{kt})")
                    inst.then_inc(sem_pe)

            emit_warm(N_WARM)
            emit_ftr(0)           # K b0 t0-7 -> banks 6/7
            emit_warm(N_WARM2)
            emit_ftr(1)           # Q h0 t0-7 -> slot 0
            emit_warm(N_WARM2)
            emit_ftr(2)           # K b0 t8-15 -> slot 1
            emit_S(0)
            emit_S(1)
            for w in range(1, NW):
                if w < NPAIR:
                    emit_S(2 * w)
                if w >= 2:
                    emit_O(w - 2)
                if w < NPAIR:
                    emit_S(2 * w + 1)

        @block.scalar
        def _(sc):
            nc.scalar.wait_ge(sem_pool, 4)
            nc.scalar.activation(                  # preload Exp table
                out=scr[:, 0:1], in_=bias0[:, 0:1],
                func=EXP, bias=bias0[:, 0:1], scale=1.0,
            )
            for g in range(2 * NPAIR):
                p = g >> 1
                half = g & 1
                slot = g % 3
                aw = aw_of(g)
                ov = PT[p % 3][:, :].rearrange("p (s c) -> p s c", c=1024)
                _lab(nc.scalar.activation(
                    out=ov[:, half:half + 1, 0:aw],
                    in_=pv[:, slot:slot + 1, 0:aw],
                    func=EXP, bias=bias0[:, 0:1], scale=SCALE,
                )._wait_ge(sem_pe, pe_after_S[g]),
                    f"exp(g{g})").then_inc(sem_act)

        @block.vector
        def _(ve):
            # fast-path cast copies psum f32 -> KT/QT f16
            for label, pe_val, dst, base in [
                ("cpK0", 8, KT[0][:, 0:1024], TR_BASE[0]),
                ("cpQ0", 16, QT[0][:, 0:1024], TR_BASE[1]),
                ("cpK1a", 20, KT[0][:, 1024:1536], TR_BASE[2]),
                ("cpK1b", 24, KT[0][:, 1536:2048], TR_BASE[2] + 512),
            ]:
                nc.vector.wait_ge(sem_pe, pe_val)
                _lab(nc.vector.tensor_copy(
                    dst, psum[:, base:base + dst.shape[1]]), label).then_inc(sem_dve)

            def emit_dexp(g):
                p = g >> 1
                half = g & 1
                slot = g % 3
                aw = aw_of(g)
                nc.vector.wait_ge(sem_pe, pe_after_S[g])
                _lab(nc.vector.tensor_scalar(
                    PTI[p % 3][:, half * 1024 + aw:half * 1024 + 1024],
                    pv[:, slot, aw:1024],
                    A16, B16, op0=mybir.AluOpType.mult,
                    op1=mybir.AluOpType.add,
                ), f"dexp(g{g})").then_inc(sem_dve)

            def emit_recip(p):
                buf = p % 2
                nc.vector.wait_ge(sem_pe, pe_after_O[p])
                if p >= 2:
                    nc.vector.wait_ge(sem_dve, mults_done[p - 2])
                _lab(nc.vector.reciprocal(
                    rsb[buf][:, 0:1], opsum(buf)[:, 128:129]),
                    f"recip({p})").then_inc(sem_dve)

            def emit_mult(p):
                h = p // NPH
                qc = p % NPH
                buf = p % 2
                nc.vector.wait_ge(sem_dve, recips_done[p])
                if qc == 0 and h >= 2:
                    nc.vector.wait_ge(sem_out[h - 2], 64)
                _lab(nc.vector.tensor_scalar(
                    OS[h % 2][:, qc * 128:(qc + 1) * 128],
                    opsum(buf)[:, 0:128],
                    rsb[buf][:, 0:1],
                    None,
                    op0=mybir.AluOpType.mult,
                ), f"mult({p})").then_inc(sem_dve)

            for op in dve_ops:
                if op[0] == "dexp":
                    emit_dexp(op[1])
                elif op[0] == "recip":
                    emit_recip(op[1])
                else:
                    emit_mult(op[1])

    return nc


_NC = None


def _get_nc():
    global _NC
    if _NC is None:
        _NC = build_attention_nc(2048, 2, 4)
    return _NC


def kernel(query, key, value):
    from concourse.bass_utils import run_bass_kernel_spmd

    query = np.ascontiguousarray(query, dtype=np.float32)
    key = np.ascontiguousarray(key, dtype=np.float32)
    value = np.ascontiguousarray(value, dtype=np.float32)
    G = query.shape[2] // key.shape[2]
    nc = _get_nc()
    in_maps = []
    for c in range(N_CORES):
        in_maps.append({
            "query": np.ascontiguousarray(query[:, :, c * G:(c + 1) * G, :]),
            "key": np.ascontiguousarray(key[:, :, c, :]),
            "value": np.ascontiguousarray(value[:, :, c, :]),
        })
    res = run_bass_kernel_spmd(nc, in_maps, list(range(N_CORES)))
    out = np.empty_like(query)
    for c in range(N_CORES):
        out[:, :, c * G:(c + 1) * G, :] = res.results[c]["out"]
    return out
